# revision 1
# baseline (speedup 1.0000x reference)
"""BitLinear on 8 TRN2 NeuronCores (Bass/Tile).

reference math:
    s      = max(|x| row)/127 (per token), clamped to EPS
    xq     = clip(round(x/s), -127, 127) * s
    gamma  = max(mean(|w|), 1e-6)
    wq     = round(clip(w/gamma, -1, 1)) * gamma
    out    = xq @ wq.T          # [8192, 4096] @ [4096, 16384]^T

Key facts exploited:
  * round(x/s) is an exact integer with |n| <= 127  -> exact in bf16.
  * round(clip(w/gamma)) is in {-1, 0, 1}           -> exact in fp8e4.
  * The integer matmul accumulates exactly in fp32 PSUM (|sum| <= 127*4096
    < 2^24), so out = (s_t*gamma) * (n @ m^T) is exact integer arithmetic
    times per-token scale -- it matches the fp32 reference up to the
    reference's own accumulation rounding (~1e-6 relative).
  * Rounding is done with the fp32 magic-number trick (+1.5*2^23 then
    subtract), which is round-half-to-even -- identical to jnp.round.

Sharding (column-parallel, per the hint): each core gets the full x
[8192, 4096] and a 2048-row weight shard pre-transposed on the host to
wt [4096, 2048]. Core c computes out[:, c*2048:(c+1)*2048].

Per-core kernel pipeline (all overlap under Tile):
  Phase W: quantize the weight shard into a resident SBUF tile
           wq[128, 32, 2048] fp8e4 (64 KiB/partition).
  Phase X (64 chunks of 128 tokens):
    DMA x chunk (2 halves of [128, 2048] f32)
    DVE absmax-reduce -> s, 1/s, s*gamma
    ACT x*(1/s)+MAGIC ; ACT -MAGIC -> bf16 integers (token-major)
    DMA-transpose (XBAR) -> xqT [128, 32, 128] (d on partitions)
    PE: 32 k-tiles x 4 psum banks of N=512 accumulating matmuls
    ACT psum * (s_t*gamma) -> sbuf, DMA out.
"""

from contextlib import ExitStack

import numpy as np

import concourse.bass as bass
import concourse.mybir as mybir
from concourse import bacc
from concourse.tile import TileContext

Q = 127.0
EPS = 1e-8
MAGIC = 12582912.0  # 1.5 * 2**23: fp32 add rounds mantissa to integer (RNE)

B, S, D, O = 4, 2048, 4096, 16384
T = B * S
NCORES = 8
O_SH = O // NCORES

F32 = mybir.dt.float32
BF16 = mybir.dt.bfloat16
FP8 = mybir.dt.float8e4


def build_program(gamma: float, t: int = T, d: int = D, o_sh: int = O_SH,
                  wq_dtype=FP8, n_free: int = 512,
                  n_reps: int = 1, use_dr: bool = False,
                  pre: int = 0, xtp_bufs: int = 6,
                  xqt_bufs: int = 4) -> bass.Bass:
    """Build the per-core Bass program (SPMD; all cores run the same code
    on their own shard). gamma is baked in as an immediate. n_reps>1 wraps
    the whole kernel in an on-device loop (for timing only).

    use_dr ('b0'|'dup'|'xp'|'splitonly'): experimental fp8 DoubleRow path
    (split n = 16a + b, both fp8e4-exact). Numerically exact and HW-correct,
    but measured ~2.4x SLOWER than the bf16 path on real trn2 (no
    double-pumping observed) — kept for reference, do not enable."""
    kt = d // 128          # contraction tiles
    mt = t // 128          # token chunks
    nb = o_sh // n_free    # psum-bank column blocks per chunk
    half = d // 2          # x is streamed in two half-rows
    kth = kt // 2
    inv_gamma = float(np.float32(1.0) / np.float32(gamma))
    inv_q = float(np.float32(1.0) / np.float32(Q))

    nc = bacc.Bacc("TRN2", target_bir_lowering=False, debug=False,
                   enable_asserts=False)
    x = nc.declare_dram_parameter("x", [t, d], F32, isOutput=False)
    wt = nc.declare_dram_parameter("wt", [d, o_sh], F32, isOutput=False)
    out = nc.declare_dram_parameter("out", [t, o_sh], F32, isOutput=True)

    with TileContext(nc) as tc, ExitStack() as ctx:
        wq_pool = ctx.enter_context(tc.tile_pool(name="wq", bufs=1))
        xtp = ctx.enter_context(tc.tile_pool(name="xtp", bufs=xtp_bufs))
        xrp = ctx.enter_context(tc.tile_pool(name="xrp", bufs=2))
        tmpp = (ctx.enter_context(tc.tile_pool(name="tmpp", bufs=2))
                if use_dr else None)
        xqp = ctx.enter_context(tc.tile_pool(name="xqp", bufs=3))
        xqt = ctx.enter_context(tc.tile_pool(name="xqt", bufs=xqt_bufs))
        osb = ctx.enter_context(tc.tile_pool(name="osb", bufs=2))
        sml = ctx.enter_context(tc.tile_pool(name="sml", bufs=6))
        psum = ctx.enter_context(tc.tile_pool(name="psum", bufs=2, space="PSUM"))
        xt8p = (ctx.enter_context(tc.tile_pool(name="xt8", bufs=3))
                if use_dr else None)

        body_cm = tc.For_i(0, n_reps, 1) if n_reps > 1 else None
        if body_cm is not None:
            body_cm.__enter__()

        dr_mode = use_dr if isinstance(use_dr, str) else ("b0" if use_dr else "")
        use_dr = bool(dr_mode)
        dr_mm = dr_mode in ("b0", "dup", "xp")

        # ---- Phase W: ternary-quantize the weight shard (resident) ----
        if dr_mode == "dup":
            wq = wq_pool.tile([128, kt, 2, o_sh], wq_dtype)
        else:
            wq = wq_pool.tile([128, kt, o_sh], wq_dtype)
        def emit_w():
            for k in range(kt):
              wstage = xtp.tile([128, o_sh], F32, tag="xt")
              nc.sync.dma_start(out=wstage[:], in_=wt[k * 128:(k + 1) * 128, :])
              wr = xrp.tile([128, o_sh], F32, tag="xr")
              # w * (1/gamma) + MAGIC  (one dual-op DVE pass)
              nc.vector.tensor_scalar(wr[:], wstage[:], inv_gamma, MAGIC,
                                      mybir.AluOpType.mult, mybir.AluOpType.add)
              wr2 = xrp.tile([128, o_sh], F32, tag="xr")
              nc.scalar.activation(wr2[:], wr[:], mybir.ActivationFunctionType.Copy,
                                   bias=-MAGIC)
              # clip to [-1, 1] and store as fp8e4 (exact for -1/0/1)
              if dr_mode == "dup":
                  nc.vector.tensor_scalar(wq[:, k, 0, :], wr2[:], 1.0, -1.0,
                                          mybir.AluOpType.min, mybir.AluOpType.max)
                  nc.scalar.activation(wq[:, k, 1, :], wq[:, k, 0, :],
                                       mybir.ActivationFunctionType.Copy)
              else:
                  nc.vector.tensor_scalar(wq[:, k, :], wr2[:], 1.0, -1.0,
                                          mybir.AluOpType.min, mybir.AluOpType.max)

        # ---- Phase X: per 128-token chunk ----
        def front_end(m):
            xts = []
            ams = []
            for h in range(2):
                xt = xtp.tile([128, half], F32, tag="xt")
                nc.sync.dma_start(
                    out=xt[:],
                    in_=x[m * 128:(m + 1) * 128, h * half:(h + 1) * half])
                am_h = sml.tile([128, 1], F32)
                nc.vector.tensor_reduce(am_h[:], xt[:], axis=mybir.AxisListType.X,
                                        op=mybir.AluOpType.max,
                                        apply_absolute_value=True)
                xts.append(xt)
                ams.append(am_h)

            am = sml.tile([128, 1], F32)
            nc.vector.tensor_tensor(am[:], ams[0][:], ams[1][:],
                                    mybir.AluOpType.max)
            s = sml.tile([128, 1], F32)
            nc.vector.tensor_scalar(s[:], am[:], inv_q, EPS,
                                    mybir.AluOpType.mult, mybir.AluOpType.max)
            rs = sml.tile([128, 1], F32)
            nc.vector.reciprocal(rs[:], s[:])
            sg = sml.tile([128, 1], F32)
            nc.vector.tensor_scalar_mul(sg[:], s[:], float(gamma))

            xqT = xqt.tile([128, kt, 128], BF16)
            for h in range(2):
                xr = xrp.tile([128, half], F32, tag="xr")
                nc.scalar.activation(xr[:], xts[h][:],
                                     mybir.ActivationFunctionType.Copy,
                                     bias=MAGIC, scale=rs[:])
                xq_h = xqp.tile([128, half], BF16)
                if use_dr:
                    # Pool takes the -MAGIC pass (1-input, line rate)
                    nc.gpsimd.tensor_scalar_add(xq_h[:], xr[:], -MAGIC)
                else:
                    nc.scalar.activation(xq_h[:], xr[:],
                                         mybir.ActivationFunctionType.Copy,
                                         bias=-MAGIC)
                nc.sync.dma_start_transpose(xqT[:, h * kth:(h + 1) * kth, :],
                                            xq_h[:])

            if use_dr:
                # split n = 16a + b in the transposed layout; a,b -> fp8
                xt8 = xt8p.tile([128, kt, 2, 128], FP8)
                for h in range(2):
                    k0, k1 = h * kth, (h + 1) * kth
                    ksl = slice(k0, k1)
                    n3 = xqT[:, ksl, :]
                    tmp = tmpp.tile([128, half], F32, tag="tmp")
                    tmp3 = tmp[:].rearrange("p (a b) -> p a b", b=128)
                    # t = n/16 + MAGIC  (n/16 is exact; +MAGIC rounds RNE)
                    nc.scalar.activation(tmp3, n3,
                                         mybir.ActivationFunctionType.Copy,
                                         bias=MAGIC, scale=0.0625)
                    if dr_mode == "xp":
                        # duo-swizzle: pair 2i=(16a_2i, b_2i+1),
                        # pair 2i+1=(b_2i, 16a_2i+1); rhs for both is the
                        # natural forward slice (w_2i, w_2i+1) — no stride-0.
                        # 16a_k -> [k, 0] (k even), [k, 1] (k odd)
                        nc.gpsimd.tensor_scalar(xt8[:, k0:k1:2, 0, :],
                                                tmp3[:, 0::2, :],
                                                16.0, -16.0 * MAGIC,
                                                mybir.AluOpType.mult,
                                                mybir.AluOpType.add)
                        nc.gpsimd.tensor_scalar(xt8[:, k0 + 1:k1:2, 1, :],
                                                tmp3[:, 1::2, :],
                                                16.0, -16.0 * MAGIC,
                                                mybir.AluOpType.mult,
                                                mybir.AluOpType.add)
                        # b_k = n_k - 16a_k -> [k+1, 0] (k even), [k-1, 1] (k odd)
                        nc.vector.scalar_tensor_tensor(
                            xt8[:, k0 + 1:k1:2, 0, :],
                            xt8[:, k0:k1:2, 0, :], -1.0, n3[:, 0::2, :],
                            mybir.AluOpType.mult, mybir.AluOpType.add)
                        nc.vector.scalar_tensor_tensor(
                            xt8[:, k0:k1:2, 1, :],
                            xt8[:, k0 + 1:k1:2, 1, :], -1.0, n3[:, 1::2, :],
                            mybir.AluOpType.mult, mybir.AluOpType.add)
                    else:
                        # 16a = t*16 - 16*MAGIC  -> fp8 (exact)
                        nc.gpsimd.tensor_scalar(xt8[:, ksl, 0, :], tmp3,
                                                16.0, -16.0 * MAGIC,
                                                mybir.AluOpType.mult,
                                                mybir.AluOpType.add)
                        # b = n - 16a -> fp8 (exact)
                        nc.vector.scalar_tensor_tensor(xt8[:, ksl, 1, :],
                                                       xt8[:, ksl, 0, :], -1.0,
                                                       n3,
                                                       mybir.AluOpType.mult,
                                                       mybir.AluOpType.add)

            else:
                xt8 = None
            return xqT, xt8, sg

        def mm_out(m, st):
            xqT, xt8, sg = st
            acc = psum.tile([128, o_sh], F32)
            if dr_mm:
                for k in range(kt):
                    lhsT = xt8[:, k, :, :]
                    for j in range(nb):
                        if dr_mode == "dup":
                            rhs = wq[:, k, :, j * n_free:(j + 1) * n_free]
                        elif dr_mode == "xp":
                            dk = 2 * (k // 2)
                            rhs = wq[:, dk:dk + 2, j * n_free:(j + 1) * n_free]
                        else:
                            rhs = (wq[:, k, j * n_free:(j + 1) * n_free]
                                   .unsqueeze(1).broadcast_to((128, 2, n_free)))
                        nc.tensor.matmul(
                            acc[:, j * n_free:(j + 1) * n_free], lhsT, rhs,
                            start=(k == 0), stop=(k == kt - 1),
                            perf_mode=mybir.MatmulPerfMode.DoubleRow)
            else:
                for k in range(kt):
                    for j in range(nb):
                        nc.tensor.matmul(
                            acc[:, j * n_free:(j + 1) * n_free],
                            xqT[:, k, :],
                            wq[:, k, j * n_free:(j + 1) * n_free],
                            start=(k == 0), stop=(k == kt - 1))

            ot = osb.tile([128, o_sh], F32)
            nc.scalar.activation(ot[:], acc[:],
                                 mybir.ActivationFunctionType.Copy,
                                 scale=sg[:])
            nc.sync.dma_start(out=out[m * 128:(m + 1) * 128, :], in_=ot[:])

        # pre>0 emits the first chunk front-ends before the weight phase so
        # their x DMAs are not queued behind the 32 MiB of weight loads.
        # The cost model likes pre=3 (-150 us startup stall) but real HW
        # measured it SLOWER (x loads starve the wq DMAs that gate every
        # early matmul k-step), so the default is pre=0.
        PRE = min(pre, mt)
        pend = {}
        for m in range(PRE):
            pend[m] = front_end(m)
        emit_w()
        for m in range(mt):
            st = pend.pop(m) if m in pend else front_end(m)
            mm_out(m, st)

        if body_cm is not None:
            body_cm.__exit__(None, None, None)

    nc.finalize()
    return nc


def _compute_gamma(weight: np.ndarray) -> float:
    g = np.mean(np.abs(weight), dtype=np.float64)
    return float(np.maximum(np.float32(g), np.float32(1e-6)))


last_run = None  # BassKernelResults of the most recent kernel() call


def kernel(x: np.ndarray, weight: np.ndarray) -> np.ndarray:
    import os

    from concourse.bass_utils import run_bass_kernel_spmd

    global last_run
    assert x.shape == (B, S, D) and weight.shape == (O, D)
    x2d = np.ascontiguousarray(x.reshape(T, D), dtype=np.float32)
    gamma = _compute_gamma(weight)

    nc = build_program(gamma)

    in_maps = []
    for c in range(NCORES):
        wt_c = np.ascontiguousarray(
            weight[c * O_SH:(c + 1) * O_SH, :].T, dtype=np.float32)
        in_maps.append({"x": x2d, "wt": wt_c})

    trace = bool(int(os.environ.get("BITLINEAR_TRACE", "0")))
    res = run_bass_kernel_spmd(nc, in_maps, list(range(NCORES)), trace=trace)
    last_run = res
    shards = [res.results[c]["out"] for c in range(NCORES)]
    full = np.concatenate(shards, axis=1).reshape(B, S, O)
    return np.asarray(full, dtype=np.float32)


if __name__ == "__main__":
    rng = np.random.default_rng(0)
    xs = rng.standard_normal((B, S, D), dtype=np.float32)
    ws = (rng.standard_normal((O, D), dtype=np.float32) * 0.02).astype(np.float32)
    o = kernel(xs, ws)
    print(o.shape, o.dtype)



# revision 12
# speedup vs baseline: 1.1888x; 1.1888x over previous
"""BitLinear on 8 TRN2 NeuronCores (Bass/Tile).

reference math:
    s      = max(|x| row)/127 (per token), clamped to EPS
    xq     = clip(round(x/s), -127, 127) * s
    gamma  = max(mean(|w|), 1e-6)
    wq     = round(clip(w/gamma, -1, 1)) * gamma
    out    = xq @ wq.T          # [8192, 4096] @ [4096, 16384]^T

Key facts exploited:
  * round(x/s) is an exact integer with |n| <= 127  -> exact in bf16.
  * round(clip(w/gamma)) is in {-1, 0, 1}           -> exact in fp8e4.
  * The integer matmul accumulates exactly in fp32 PSUM (|sum| <= 127*4096
    < 2^24), so out = (s_t*gamma) * (n @ m^T) is exact integer arithmetic
    times per-token scale -- it matches the fp32 reference up to the
    reference's own accumulation rounding (~1e-6 relative).
  * Rounding is done with the fp32 magic-number trick (+1.5*2^23 then
    subtract), which is round-half-to-even -- identical to jnp.round.

Sharding (column-parallel, per the hint): each core gets the full x
[8192, 4096] and a 2048-row weight shard pre-transposed on the host to
wt [4096, 2048]. Core c computes out[:, c*2048:(c+1)*2048].

Per-core kernel pipeline (all overlap under Tile):
  Phase W: quantize the weight shard into a resident SBUF tile
           wq[128, 32, 2048] fp8e4 (64 KiB/partition).
  Phase X (64 chunks of 128 tokens):
    DMA x chunk (2 halves of [128, 2048] f32)
    DVE absmax-reduce -> s, 1/s, s*gamma
    ACT x*(1/s)+MAGIC ; GPSIMD -MAGIC -> bf16 integers (token-major)
    DMA-transpose (XBAR) -> xqT [128, 32, 128] (d on partitions)
    PE: 32 k-tiles x 4 psum banks of N=512 accumulating matmuls
    ACT psum * (s_t*gamma) -> sbuf, DMA out.

Measured (this machine, 8 cores busy): PE streams ~0.55 ns/col (an
effective ~1.8 GHz power state; the 2.4 GHz nominal would be 0.417).
The kernel is PE-bound; per-rep weight re-quant in the timing loop
costs ~116 us unless overlapped across reps:
  w_mode "in"     : weight phase at loop-body head (original; stalls PE
                    at each rep boundary -- wq is single-buffered).
  w_mode "tail"   : weight phase at body tail, quantizing for the NEXT
                    rep; partial overlap with the matmul tail.
  w_mode "unroll2": two wq buffers, body covers 2 reps; weight loads of
                    rep i+1 fully overlap rep i's matmuls.
Perf notes from microbenchmarks (see transcript): fp8 DoubleRow gives 2x
virtual contraction per column but x needs a 16a+b split into two fp8
components, which exactly cancels the gain -- dead end. N>512 and bf16
PSUM are rejected by bass/walrus. --enable-ldw-opt=true crashes walrus.
"""

from contextlib import ExitStack

import numpy as np

import concourse.bass as bass
import concourse.mybir as mybir
from concourse import bacc
from concourse.tile import TileContext

Q = 127.0
EPS = 1e-8
MAGIC = 12582912.0  # 1.5 * 2**23: fp32 add rounds mantissa to integer (RNE)

B, S, D, O = 4, 2048, 4096, 16384
T = B * S
NCORES = 8
O_SH = O // NCORES

F32 = mybir.dt.float32
BF16 = mybir.dt.bfloat16
FP8 = mybir.dt.float8e4


def build_program(gamma: float, t: int = T, d: int = D, o_sh: int = O_SH,
                  n_free: int = 512, n_reps: int = 1,
                  w_mode: str = "unroll2", pool_neg: bool = False,
                  xtp_bufs: int = 2, xqt_bufs: int = 2, xqp_bufs: int = 1,
                  xrp_bufs: int = 1, sml_bufs: int = 6,
                  wtp_bufs: int = 1, wrp_bufs: int = 2, osb_bufs: int = 1,
                  pre: int = 0) -> bass.Bass:
    """Build the per-core Bass program (SPMD; all cores run the same code
    on their own shard). gamma is baked in as an immediate. n_reps>1 wraps
    the kernel in an on-device loop (for timing only)."""
    kt = d // 128          # contraction tiles
    mt = t // 128          # token chunks
    nb = o_sh // n_free    # psum-bank column blocks per chunk
    half = d // 2          # x is streamed in two half-rows
    kth = kt // 2
    inv_gamma = float(np.float32(1.0) / np.float32(gamma))
    inv_q = float(np.float32(1.0) / np.float32(Q))

    if w_mode == "unroll2" and n_reps > 1 and n_reps % 2 != 0:
        w_mode = "tail"  # unroll2 needs even n_reps; tail is the fallback

    nc = bacc.Bacc("TRN2", target_bir_lowering=False, debug=False,
                   enable_asserts=False)
    x = nc.declare_dram_parameter("x", [t, d], F32, isOutput=False)
    wt = nc.declare_dram_parameter("wt", [d, o_sh], F32, isOutput=False)
    out = nc.declare_dram_parameter("out", [t, o_sh], F32, isOutput=True)

    with TileContext(nc) as tc, ExitStack() as ctx:
        # wq tiles are persistent and manually alternated (unroll2 allocates
        # two distinct tiles from this single-buffered pool).
        wq_pool = ctx.enter_context(tc.tile_pool(name="wq", bufs=1))
        xtp = ctx.enter_context(tc.tile_pool(name="xtp", bufs=xtp_bufs))
        xrp = ctx.enter_context(tc.tile_pool(name="xrp", bufs=xrp_bufs))
        xqp = ctx.enter_context(tc.tile_pool(name="xqp", bufs=xqp_bufs))
        xqt = ctx.enter_context(tc.tile_pool(name="xqt", bufs=xqt_bufs))
        osb = ctx.enter_context(tc.tile_pool(name="osb", bufs=osb_bufs))
        sml = ctx.enter_context(tc.tile_pool(name="sml", bufs=sml_bufs))
        psum = ctx.enter_context(tc.tile_pool(name="psum", bufs=2, space="PSUM"))
        # Dedicated weight-phase staging pools. Sharing xtp/xrp with the
        # front-end makes the W-phase's 32 staging allocations precede the
        # next rep's front-end allocations in pool order, serializing the
        # rep boundary for ~128 us of PE idle (seen in TimelineSim).
        wtp = (ctx.enter_context(tc.tile_pool(name="wtp", bufs=wtp_bufs))
               if wtp_bufs else None)
        wrp = (ctx.enter_context(tc.tile_pool(name="wrp", bufs=wrp_bufs))
               if wrp_bufs else None)

        # ---- Phase W: ternary-quantize the weight shard into wq ----
        def emit_w(wq):
            for k in range(kt):
                if wtp is not None:
                    wstage = wtp.tile([128, o_sh], F32, tag="wt")
                else:
                    wstage = xtp.tile([128, o_sh], F32, tag="xt")
                nc.sync.dma_start(out=wstage[:],
                                  in_=wt[k * 128:(k + 1) * 128, :])
                if wrp is not None:
                    wr = wrp.tile([128, o_sh], F32, tag="wr")
                else:
                    wr = xrp.tile([128, o_sh], F32, tag="xr")
                # w * (1/gamma) + MAGIC  (one dual-op DVE pass)
                nc.vector.tensor_scalar(wr[:], wstage[:], inv_gamma, MAGIC,
                                        mybir.AluOpType.mult,
                                        mybir.AluOpType.add)
                if wrp is not None:
                    wr2 = wrp.tile([128, o_sh], F32, tag="wr")
                else:
                    wr2 = xrp.tile([128, o_sh], F32, tag="xr")
                if pool_neg:
                    nc.gpsimd.tensor_scalar_add(wr2[:], wr[:], -MAGIC)
                else:
                    nc.scalar.activation(wr2[:], wr[:],
                                         mybir.ActivationFunctionType.Copy,
                                         bias=-MAGIC)
                # clip to [-1, 1] and store as fp8e4 (exact for -1/0/1)
                nc.vector.tensor_scalar(wq[:, k, :], wr2[:], 1.0, -1.0,
                                        mybir.AluOpType.min,
                                        mybir.AluOpType.max)

        # ---- Phase X: per 128-token chunk ----
        def front_end(m):
            xts = []
            ams = []
            for h in range(2):
                xt = xtp.tile([128, half], F32, tag="xt")
                nc.sync.dma_start(
                    out=xt[:],
                    in_=x[m * 128:(m + 1) * 128, h * half:(h + 1) * half])
                am_h = sml.tile([128, 1], F32)
                nc.vector.tensor_reduce(am_h[:], xt[:],
                                        axis=mybir.AxisListType.X,
                                        op=mybir.AluOpType.max,
                                        apply_absolute_value=True)
                xts.append(xt)
                ams.append(am_h)

            am = sml.tile([128, 1], F32)
            nc.vector.tensor_tensor(am[:], ams[0][:], ams[1][:],
                                    mybir.AluOpType.max)
            s = sml.tile([128, 1], F32)
            nc.vector.tensor_scalar(s[:], am[:], inv_q, EPS,
                                    mybir.AluOpType.mult, mybir.AluOpType.max)
            rs = sml.tile([128, 1], F32)
            nc.vector.reciprocal(rs[:], s[:])
            sg = sml.tile([128, 1], F32)
            nc.vector.tensor_scalar_mul(sg[:], s[:], float(gamma))

            xqT = xqt.tile([128, kt, 128], BF16)
            for h in range(2):
                xr = xrp.tile([128, half], F32, tag="xr")
                nc.scalar.activation(xr[:], xts[h][:],
                                     mybir.ActivationFunctionType.Copy,
                                     bias=MAGIC, scale=rs[:])
                xq_h = xqp.tile([128, half], BF16)
                if pool_neg:
                    # Pool takes the -MAGIC pass (1-input, line rate)
                    nc.gpsimd.tensor_scalar_add(xq_h[:], xr[:], -MAGIC)
                else:
                    nc.scalar.activation(xq_h[:], xr[:],
                                         mybir.ActivationFunctionType.Copy,
                                         bias=-MAGIC)
                nc.sync.dma_start_transpose(xqT[:, h * kth:(h + 1) * kth, :],
                                            xq_h[:])
            return xqT, sg

        def mm_out(m, st, wq):
            xqT, sg = st
            acc = psum.tile([128, o_sh], F32)
            for k in range(kt):
                for j in range(nb):
                    nc.tensor.matmul(
                        acc[:, j * n_free:(j + 1) * n_free],
                        xqT[:, k, :],
                        wq[:, k, j * n_free:(j + 1) * n_free],
                        start=(k == 0), stop=(k == kt - 1))

            ot = osb.tile([128, o_sh], F32)
            nc.scalar.activation(ot[:], acc[:],
                                 mybir.ActivationFunctionType.Copy,
                                 scale=sg[:])
            nc.sync.dma_start(out=out[m * 128:(m + 1) * 128, :], in_=ot[:])

        def chunks(wq):
            PRE = min(pre, mt)
            pend = {}
            for m in range(PRE):
                pend[m] = front_end(m)
            for m in range(mt):
                st = pend.pop(m) if m in pend else front_end(m)
                mm_out(m, st, wq)

        if n_reps == 1:
            wq = wq_pool.tile([128, kt, o_sh], FP8)
            emit_w(wq)
            chunks(wq)
        elif w_mode == "in":
            wq = wq_pool.tile([128, kt, o_sh], FP8)
            with tc.For_i(0, n_reps, 1):
                emit_w(wq)
                chunks(wq)
        elif w_mode == "tail_py":
            # Python-unrolled tail mode (TimelineSim can't follow For_i)
            wq = wq_pool.tile([128, kt, o_sh], FP8)
            emit_w(wq)
            for _ in range(n_reps):
                chunks(wq)
                emit_w(wq)
        elif w_mode == "tail":
            wq = wq_pool.tile([128, kt, o_sh], FP8)
            emit_w(wq)  # preamble: first rep's weights
            with tc.For_i(0, n_reps, 1):
                chunks(wq)
                emit_w(wq)  # quantize for the next rep (tail overlap)
        elif w_mode == "unroll2":
            wqA = wq_pool.tile([128, kt, o_sh], FP8)
            wqB = wq_pool.tile([128, kt, o_sh], FP8)
            emit_w(wqA)  # preamble
            with tc.For_i(0, n_reps // 2, 1):
                emit_w(wqB)   # overlaps chunks(wqA) fully (indep. buffers)
                chunks(wqA)
                emit_w(wqA)   # overlaps chunks(wqB); next iter reads wqA
                chunks(wqB)
        elif w_mode == "u2_py":
            wqA = wq_pool.tile([128, kt, o_sh], FP8)
            wqB = wq_pool.tile([128, kt, o_sh], FP8)
            emit_w(wqA)
            for _ in range(n_reps // 2):
                emit_w(wqB)
                chunks(wqA)
                emit_w(wqA)
                chunks(wqB)
        else:
            raise ValueError(w_mode)

    nc.finalize()
    return nc


def _compute_gamma(weight: np.ndarray) -> float:
    g = np.mean(np.abs(weight), dtype=np.float64)
    return float(np.maximum(np.float32(g), np.float32(1e-6)))


last_run = None  # BassKernelResults of the most recent kernel() call


def kernel(x: np.ndarray, weight: np.ndarray) -> np.ndarray:
    import os

    from concourse.bass_utils import run_bass_kernel_spmd

    global last_run
    assert x.shape == (B, S, D) and weight.shape == (O, D)
    x2d = np.ascontiguousarray(x.reshape(T, D), dtype=np.float32)
    gamma = _compute_gamma(weight)

    nc = build_program(gamma)

    in_maps = []
    for c in range(NCORES):
        wt_c = np.ascontiguousarray(
            weight[c * O_SH:(c + 1) * O_SH, :].T, dtype=np.float32)
        in_maps.append({"x": x2d, "wt": wt_c})

    trace = bool(int(os.environ.get("BITLINEAR_TRACE", "0")))
    res = run_bass_kernel_spmd(nc, in_maps, list(range(NCORES)), trace=trace)
    last_run = res
    shards = [res.results[c]["out"] for c in range(NCORES)]
    full = np.concatenate(shards, axis=1).reshape(B, S, O)
    return np.asarray(full, dtype=np.float32)


if __name__ == "__main__":
    rng = np.random.default_rng(0)
    xs = rng.standard_normal((B, S, D), dtype=np.float32)
    ws = (rng.standard_normal((O, D), dtype=np.float32) * 0.02).astype(np.float32)
    o = kernel(xs, ws)
    print(o.shape, o.dtype)
